# revision 25
# baseline (speedup 1.0000x reference)
"""Trainium2 Bass kernel for the Koopman DEINA model.

Computation (per reference):
  encoder: h1 = relu(x W0^T + b0); h2 = relu(h1 W1^T + b1); g = h2 W2^T
  y2s      = concat(xs[:,1:], g[:,1:])                       [B,127,80]
  y2s_pred = scan: p_{n+1} = K p_n + Bw u_n, p_0 = [x_0; g_0], output p_1..p_127

Strategy (8 NeuronCores, data-parallel over batch, 256 batch elems/core):
  - Encoder in bf16 with fp32 psum; tokens processed t-major (token = t*256+b),
    x PE-transposed in 512-token tiles (4 groups x 32 cols, 16 zero-weight pad).
  - PE pipeline continuity is the design driver: matmuls are emitted so the
    array never waits on a drain (measured: back-to-back matmuls stream at
    ~0.42ns/row with weight loads hidden; dependency breaks cost ~2x).
  - y2s x-part: two whole-tensor DRAM->DRAM DMAs (no SBUF hop, no copies).
  - y2s g-part: f32 PE transposes into PSUM, DMA'd straight from PSUM.
  - pred: chunked closed form (S=8), phase-C outputs DMA'd straight from PSUM.
  - u transposes and the 15-step boundary chain are interleaved into the
    encoder superblock loop so their latency hides under encoder matmuls.
"""
import os
import numpy as np

DIM = 16
H1 = H2 = 256
G = 64
L = 80          # DIM + G
T = 128
NB = 2048       # full batch
NCORES = 8
BC = NB // NCORES   # 256 batch elems per core
S = 8           # recurrence chunk size
NCH = T // S    # 16 chunks
NTOK = BC * T   # 32768 tokens per core
NSB = NTOK // 2048   # 16 superblocks of 2048 tokens (= 8 t-steps x 256 b)

# f32r const tile column offsets (recurrence)
OFF_RA = 0        # [80, 640]   R_A[k,(s-1)*80+l] = (K^s)[l,k]
OFF_RU = 640      # [128, 640]  R_U[j*16+d,(s-1)*80+l] = (K^{s-1-j} Bw)[l,d], j<s
OFF_RX = 1280     # [16, 640]   R_X[d,(s-1)*80+l] = (K^s)[l,d]
OFF_RG = 1920     # [128, 1280] R_G[kp, kc*640+(s-1)*80+l] = (K^s[:,16:] @ W2)[l, kc*128+kp]
OFF_W0R = 3200    # [16, 256]   exact W0T (f32r) for the h2_0 recompute
OFF_W1R = 3456    # [128, 512]  exact W1T (f32r)
CW_COLS = 3968
# bf16 const tile column offsets (encoder)
OFF_W0 = 0        # [128, 256]  W0T on 4x32-row groups (rows 0:16 of each)
OFF_W0S = 256     # [128, 256]  shifted variant (rows 16:32 of each group)
OFF_W1 = 512      # [128, 512]  W1T, kc blocks of 256 cols
OFF_W2 = 1024     # [128, 128]  W2T, kc blocks of 64 cols
CWH_COLS = 1152

_BUILT = None


def _precompute_consts(W0, b0, W1, b1, W2, Bw, K):
    """Host-side weight folding: CW [128,CW_COLS] f32(r), CWH [128,CWH_COLS] bf16,
    CB [128,4] f32."""
    import ml_dtypes
    W0 = np.asarray(W0, np.float64)
    W1 = np.asarray(W1, np.float64)
    W2 = np.asarray(W2, np.float64)
    Bw = np.asarray(Bw, np.float64)
    K = np.asarray(K, np.float64)

    CWH = np.zeros((128, CWH_COLS), np.float64)
    for g in range(4):
        CWH[g * 32:g * 32 + 16, OFF_W0:OFF_W0 + 256] = W0.T
        CWH[g * 32 + 16:g * 32 + 32, OFF_W0S:OFF_W0S + 256] = W0.T
    for kc in range(2):
        CWH[:, OFF_W1 + kc * 256:OFF_W1 + (kc + 1) * 256] = W1[:, kc * 128:(kc + 1) * 128].T
        CWH[:, OFF_W2 + kc * 64:OFF_W2 + (kc + 1) * 64] = W2[:, kc * 128:(kc + 1) * 128].T

    CW = np.zeros((128, CW_COLS), np.float64)
    CW[0:16, OFF_W0R:OFF_W0R + 256] = W0.T
    for kc in range(2):
        CW[:, OFF_W1R + kc * 256:OFF_W1R + (kc + 1) * 256] = W1[:, kc * 128:(kc + 1) * 128].T
    A = [np.eye(L)]
    for _ in range(S):
        A.append(K @ A[-1])
    Dm = [Bw]
    for _ in range(S - 1):
        Dm.append(K @ Dm[-1])
    for s in range(1, S + 1):
        col = (s - 1) * L
        CW[0:L, OFF_RA + col:OFF_RA + col + L] = A[s].T
        CW[0:16, OFF_RX + col:OFF_RX + col + L] = A[s][:, :16].T
        AsG = A[s][:, 16:] @ W2          # [80, 256]
        for kc in range(2):
            CW[:, OFF_RG + kc * 640 + col:OFF_RG + kc * 640 + col + L] = \
                AsG[:, kc * 128:(kc + 1) * 128].T
        for j in range(s):
            CW[j * 16:(j + 1) * 16, OFF_RU + col:OFF_RU + col + L] = Dm[s - 1 - j].T

    CB = np.zeros((128, 4), np.float64)
    CB[:, 0] = np.asarray(b0)[0:128]
    CB[:, 1] = np.asarray(b0)[128:256]
    CB[:, 2] = np.asarray(b1)[0:128]
    CB[:, 3] = np.asarray(b1)[128:256]
    return (CW.astype(np.float32), CWH.astype(ml_dtypes.bfloat16),
            CB.astype(np.float32))


def _build():
    import concourse.bass as bass
    import concourse.bacc as bacc
    import concourse.mybir as mybir
    import concourse.tile as tile
    from concourse.masks import make_identity

    f32 = mybir.dt.float32
    f32r = mybir.dt.float32r
    bf16 = mybir.dt.bfloat16
    AF = mybir.ActivationFunctionType
    ALU = mybir.AluOpType
    P = 128

    nc = bacc.Bacc(None, target_bir_lowering=False, debug=False)

    xs_d = nc.dram_tensor("xs", [BC, T, DIM], f32, kind="ExternalInput")
    us_d = nc.dram_tensor("us", [BC, T, DIM], f32, kind="ExternalInput")
    cw_d = nc.dram_tensor("CW", [P, CW_COLS], f32r, kind="ExternalInput")
    cwh_d = nc.dram_tensor("CWH", [P, CWH_COLS], bf16, kind="ExternalInput")
    cb_d = nc.dram_tensor("CB", [P, 4], f32, kind="ExternalInput")
    y2s_d = nc.dram_tensor("y2s", [BC, T - 1, L], f32, kind="ExternalOutput")
    pred_d = nc.dram_tensor("y2s_pred", [BC, T - 1, L], f32, kind="ExternalOutput")

    xs_ap = xs_d.ap()
    us2d = us_d.ap().rearrange("b t d -> b (t d)")     # [256, 2048]

    with tile.TileContext(nc) as tc:
        with (
            tc.tile_pool(name="consts", bufs=1) as consts,
            tc.tile_pool(name="store", bufs=1) as store,
            tc.tile_pool(name="ld", bufs=6) as ld,
            tc.tile_pool(name="xT", bufs=4) as xTp,
            tc.tile_pool(name="h1sb", bufs=3) as h1sbp,
            tc.tile_pool(name="h2sb", bufs=3) as h2sbp,
            tc.tile_pool(name="osb", bufs=6) as osbp,
        ):
            ident = consts.tile([P, P], f32)
            make_identity(nc, ident[:])
            identh = consts.tile([P, P], bf16)
            make_identity(nc, identh[:])
            CWH = consts.tile([P, CWH_COLS], bf16)
            nc.sync.dma_start(CWH[:], cwh_d[:])
            CB = consts.tile([P, 4], f32)
            nc.sync.dma_start(CB[:], cb_d[:])
            CW = consts.tile([P, CW_COLS], f32r)

            uT = store.tile([P, NCH * BC], f32r)       # [(j,d), c*256 + b]
            p8 = store.tile([80, NCH * BC], f32r)      # block c = p_{8c} (c>=1)
            x0T = store.tile([16, BC], f32r)
            h10 = store.tile([P, 2, BC], f32r)
            h2_0 = store.tile([P, 2, BC], f32r)


            def pool_copy(dst, src, alt=None):
                # psum-sourced copy: Pool/GpSimd cannot read PSUM, so these
                # go to DVE (vector) or ACT (scalar)
                e = alt or nc.vector
                if e is nc.scalar:
                    e.copy(dst, src)
                else:
                    e.tensor_copy(dst, src)

            def cwr(r0, r1, c0, c1):
                return CW[r0:r1, c0:c1]

            def cwh(r0, r1, c0, c1):
                return CWH[r0:r1, c0:c1]


            # ---------------- x0 transpose ----------------
            with tc.tile_pool(name="x0ps", bufs=2,
                              space=bass.MemorySpace.PSUM) as x0ps:
                for bt in range(2):
                    x0l = ld.tile([P, 16], f32, tag="x0l")
                    nc.sync.dma_start(x0l[:], xs_ap[bt * 128:(bt + 1) * 128, 0, :])
                    tp = x0ps.tile([P, 512], f32, tag="x0")
                    nc.tensor.transpose(tp[0:16, 0:128], x0l[:], ident[:])
                    nc.vector.tensor_copy(x0T[:, bt * 128:(bt + 1) * 128],
                                          tp[0:16, 0:128])

            # ---------------- chain + u-transpose helpers ----------------
            def chain_step(c, pool):
                # p8[c+1] = A8 p8[c] + sum_j K^{7-j} Bw u_{8c+j}  (s=8 col block)
                wp = pool.tile([P, 512], f32, tag="l2", name=f"wp{c}")
                nc.tensor.matmul(wp[0:80, 0:BC], cwr(0, 128, OFF_RU + 560, OFF_RU + 640),
                                 uT[:, c * BC:(c + 1) * BC], start=True, stop=False)
                if c == 0:
                    nc.tensor.matmul(wp[0:80, 0:BC], cwr(0, 16, OFF_RX + 560, OFF_RX + 640),
                                     x0T[:], start=False, stop=False)
                    for kc in range(2):
                        nc.tensor.matmul(wp[0:80, 0:BC],
                                         cwr(0, 128, OFF_RG + kc * 640 + 560, OFF_RG + kc * 640 + 640),
                                         h2_0[:, kc, :], start=False, stop=(kc == 1))
                else:
                    nc.tensor.matmul(wp[0:80, 0:BC], cwr(0, 80, OFF_RA + 560, OFF_RA + 640),
                                     p8[:, c * BC:(c + 1) * BC], start=False, stop=True)
                pool_copy(p8[:, (c + 1) * BC:(c + 2) * BC], wp[0:80, 0:BC])

            def u_transpose(au, sb, j, pool):
                # j in 0..3 -> (kk, bt); k = 2*sb + kk
                kk, bt = j // 2, j % 2
                k = 2 * sb + kk
                tp = pool.tile([P, 512], f32, tag="l2", name=f"ut{sb}_{j}")
                nc.tensor.transpose(tp[:, 0:128], au[:, bt, kk * 128:(kk + 1) * 128],
                                    ident[:])
                pool_copy(uT[:, k * 256 + bt * 128:k * 256 + (bt + 1) * 128],
                          tp[:, 0:128], alt=nc.scalar)

            with (
                tc.tile_pool(name="mixb", bufs=2, space=bass.MemorySpace.PSUM) as mixb,
                tc.tile_pool(name="l3ps", bufs=2, space=bass.MemorySpace.PSUM) as l3ps,
                tc.tile_pool(name="h1ps", bufs=2, space=bass.MemorySpace.PSUM) as h1ps,
                tc.tile_pool(name="l2ps", bufs=2, space=bass.MemorySpace.PSUM) as l2ps,
            ):
                # ---------------- encoder superblocks ----------------
                for sb in range(NSB):
                    # x gather: per bt-half, each partition's 8 (or 9)
                    # timesteps are one contiguous 576B run -> 128 descriptors
                    # per DMA (descriptor generation on Sync is the scarce
                    # resource). The PE transposes then read overlapping
                    # 32-col windows (16 real + 16 zero-weighted pad) via a
                    # custom AP. ax[p, bt*144 + j] = xs[bt*128+p, 8sb + j//16, j%16]
                    ax = ld.tile([P, 288], f32, tag="ax")
                    last_sb = sb == NSB - 1
                    nld = 128 if last_sb else 144
                    for bt in range(2):
                        in_ap = bass.AP(xs_d, bt * 128 * 2048 + 8 * sb * 16,
                                        [[2048, 128], [1, nld]])
                        nc.sync.dma_start(ax[:, bt * 144:bt * 144 + nld], in_ap)
                    if last_sb:
                        # cols 272:288 of this rotating buffer are stale (the
                        # 128-elem load stops at t=127); overwrite with finite
                        # data so the zero-weighted pad cannot be NaN garbage
                        nc.gpsimd.tensor_copy(ax[:, 272:288], ax[:, 0:16])
                    # one DVE pass builds the 4x(tt,g0) overlapping 32-col
                    # windows in bf16 (the PE rejects overlapping-window
                    # stationary APs; the DVE does not care)
                    axp = ax[:]
                    axw = xTp.tile([P, 512], bf16, tag="axw")
                    srcw = bass.AP(axp.tensor, axp.offset,
                                   [[axp.ap[0][0], 128], [32, 4], [16, 2],
                                    [144, 2], [1, 32]])
                    nc.vector.tensor_copy(
                        axw[:].rearrange("p (a b c d) -> p a b c d",
                                         a=4, b=2, c=2), srcw)
                    xts = xTp.tile([P, 512], bf16, tag="xT")
                    for i in range(4):
                        tp = mixb.tile([P, 512], bf16, tag="mixb")
                        nc.tensor.transpose(tp[:, 0:128],
                                            axw[:, i * 128:(i + 1) * 128],
                                            identh[:])
                        nc.vector.tensor_copy(xts[:, i * 128:(i + 1) * 128],
                                              tp[:, 0:128])

                    # u loads for sb<8 (transposes interleaved below)
                    au = None
                    if sb < 8:
                        au = ld.tile([P, 2, 256], f32, tag="au")
                        for bt in range(2):
                            nc.sync.dma_start(
                                au[:, bt, :],
                                us2d[bt * 128:(bt + 1) * 128,
                                     2 * sb * 128:(2 * sb + 2) * 128])
                    if sb == 0:
                        # 2MB recurrence consts: queued after sb0's loads so
                        # the first matmuls aren't stuck behind it; first use
                        # is h2_0 at sb1
                        nc.sync.dma_start(CW[:], cw_d[:])

                    h1s = [h1sbp.tile([P, 2048], bf16, tag=f"h1sb{mc}", name=f"h1s{mc}")
                           for mc in range(2)]
                    h2s = [h2sbp.tile([P, 2048], bf16, tag=f"h2sb{mc}", name=f"h2s{mc}")
                           for mc in range(2)]
                    gps = [l3ps.tile([P, 512], f32, tag="l3", name=f"gps{h}")
                           for h in range(2)]

                    def emit_l1(g):
                        for mc in range(2):
                            hp = h1ps.tile([P, 512], f32, tag="h1", name="hp")
                            nc.tensor.matmul(hp[:],
                                             cwh(g * 32, (g + 1) * 32, OFF_W0 + mc * 128, OFF_W0 + (mc + 1) * 128),
                                             xts[g * 32:(g + 1) * 32, :],
                                             start=True, stop=True,
                                             tile_position=(32 * g, 0))
                            if mc == 0:
                                nc.scalar.activation(
                                    h1s[mc][:, g * 512:(g + 1) * 512], hp[:],
                                    AF.Relu, bias=CB[:, mc:mc + 1])
                            else:
                                nc.vector.tensor_scalar(
                                    h1s[mc][:, g * 512:(g + 1) * 512], hp[:],
                                    CB[:, mc:mc + 1], 0.0,
                                    op0=ALU.add, op1=ALU.max)

                    def emit_l2(nb):
                        for mc in range(2):
                            lp = l2ps.tile([P, 512], f32, tag="l2", name="lp")
                            for kc in range(2):
                                nc.tensor.matmul(lp[:],
                                                 cwh(0, 128, OFF_W1 + kc * 256 + mc * 128,
                                                     OFF_W1 + kc * 256 + (mc + 1) * 128),
                                                 h1s[kc][:, nb * 512:(nb + 1) * 512],
                                                 start=(kc == 0), stop=(kc == 1))
                            if mc == 0:
                                nc.vector.tensor_scalar(
                                    h2s[mc][:, nb * 512:(nb + 1) * 512], lp[:],
                                    CB[:, 2 + mc:3 + mc], 0.0,
                                    op0=ALU.add, op1=ALU.max)
                            else:
                                nc.scalar.activation(
                                    h2s[mc][:, nb * 512:(nb + 1) * 512], lp[:],
                                    AF.Relu, bias=CB[:, 2 + mc:3 + mc])

                    def emit_l3(nb):
                        # flipped: out[token, gfeat] directly (no transpose)
                        h = nb % 2
                        for i in range(4):
                            tq = 2 * i + nb // 2
                            bk = nb * 4 + i
                            for kc in range(2):
                                nc.tensor.matmul(
                                    gps[h][:, tq * 64:(tq + 1) * 64],
                                    h2s[kc][:, bk * 128:(bk + 1) * 128],
                                    cwh(0, 128, OFF_W2 + kc * 64, OFF_W2 + (kc + 1) * 64),
                                    start=(kc == 0), stop=(kc == 1))

                    t_lo = 1 if sb == 0 else 0

                    def emit_out(h):
                        # gps[h] psum is already [p, (t',l)]: hop to SBUF
                        # (DMA cannot read PSUM), DMA into y2s[.., 16:80]
                        gout = osbp.tile([P, 512], f32, tag=f"gout{h}")
                        pool_copy(gout[:], gps[h][:],
                                  alt=nc.vector if h == 0 else nc.scalar)
                        src = gout[:, t_lo * 64:512].rearrange("p (t l) -> p t l", l=64)
                        nc.gpsimd.dma_start(
                            y2s_d[h * 128:(h + 1) * 128,
                                  8 * sb - 1 + t_lo:8 * sb + 7, 16:80],
                            src)

                    def maybe_u(j):
                        if au is not None:
                            u_transpose(au, sb, j, l2ps)

                    def maybe_chain(j):
                        # chain step c emitted at sb 4..11, 2 per sb
                        if 4 <= sb <= 11:
                            c = 2 * (sb - 4) + j
                            if c < NCH - 1:
                                chain_step(c, l2ps)

                    if sb == 1:
                        # h1_0 / h2_0 exact recompute (f32r) for chain/phase C;
                        # here so the 2MB CW load has drained off the queues
                        for mc in range(2):
                            hp = h1ps.tile([P, 512], f32, tag="h1",
                                           name=f"h10ps{mc}")
                            nc.tensor.matmul(
                                hp[:, 0:256],
                                cwr(0, 16, OFF_W0R + mc * 128, OFF_W0R + (mc + 1) * 128),
                                x0T[:], start=True, stop=True)
                            nc.scalar.activation(h10[:, mc, :], hp[:, 0:256],
                                                 AF.Relu, bias=CB[:, mc:mc + 1])
                        for mc in range(2):
                            lp = l2ps.tile([P, 512], f32, tag="l2",
                                           name=f"h20ps{mc}")
                            for kc in range(2):
                                nc.tensor.matmul(
                                    lp[:, 0:256],
                                    cwr(0, 128, OFF_W1R + kc * 256 + mc * 128,
                                        OFF_W1R + kc * 256 + (mc + 1) * 128),
                                    h10[:, kc, :], start=(kc == 0), stop=(kc == 1))
                            nc.vector.tensor_scalar(h2_0[:, mc, :], lp[:, 0:256],
                                                    CB[:, 2 + mc:3 + mc], 0.0,
                                                    op0=ALU.add, op1=ALU.max)
                    emit_l1(0)
                    emit_l1(1)
                    maybe_u(0)
                    emit_l1(2)
                    emit_l2(0)
                    maybe_u(1)
                    maybe_chain(0)
                    emit_l1(3)
                    emit_l2(1)
                    emit_l3(0)
                    maybe_u(2)
                    emit_l2(2)
                    emit_l3(1)
                    maybe_u(3)
                    emit_l2(3)
                    emit_l3(2)
                    emit_out(0)
                    maybe_chain(1)
                    emit_l3(3)
                    emit_out(1)

            # y2s x-part: DRAM -> DRAM, emitted after the encoder so its
            # ~32k small packets drain during phase C instead of blocking the
            # startup loads. y2s[b, tau, 0:16] = xs[b, tau+1, :], tau in [0,127)
            for bh in range(2):
                for qt in range(4):
                    t0 = qt * 32 - (1 if qt > 0 else 0)
                    nt = 32 if qt > 0 else 31
                    in_ap = bass.AP(xs_d,
                                    bh * 128 * 2048 + (t0 + 1) * 16,
                                    [[2048, 128], [16, nt], [1, 16]])
                    out_ap = bass.AP(y2s_d,
                                     bh * 128 * 127 * 80 + t0 * 80,
                                     [[127 * 80, 128], [80, nt], [1, 16]])
                    nc.gpsimd.dma_start(out_ap, in_ap)

            # ---------------- phase C: chunk outputs, DMA from PSUM ----------------
            with tc.tile_pool(name="oC", bufs=4, space=bass.MemorySpace.PSUM) as oC:
                engs = [nc.sync, nc.scalar, nc.sync, nc.scalar]
                for c in range(NCH):
                    for bt in range(2):
                        op = oC.tile([P, 2, 512], f32, tag="oC")
                        for hh in range(2):
                            if c == 0:
                                nc.tensor.matmul(op[:, hh, 0:320],
                                                 x0T[:, bt * 128:(bt + 1) * 128],
                                                 cwr(0, 16, OFF_RX + hh * 320, OFF_RX + (hh + 1) * 320),
                                                 start=True, stop=False)
                                for kc in range(2):
                                    nc.tensor.matmul(op[:, hh, 0:320],
                                                     h2_0[:, kc, bt * 128:(bt + 1) * 128],
                                                     cwr(0, 128, OFF_RG + kc * 640 + hh * 320,
                                                         OFF_RG + kc * 640 + (hh + 1) * 320),
                                                     start=False, stop=False)
                            else:
                                nc.tensor.matmul(op[:, hh, 0:320],
                                                 p8[:, c * BC + bt * 128:c * BC + (bt + 1) * 128],
                                                 cwr(0, 80, OFF_RA + hh * 320, OFF_RA + (hh + 1) * 320),
                                                 start=True, stop=False)
                            nc.tensor.matmul(op[:, hh, 0:320],
                                             uT[:, c * BC + bt * 128:c * BC + (bt + 1) * 128],
                                             cwr(0, 128, OFF_RU + hh * 320, OFF_RU + (hh + 1) * 320),
                                             start=False, stop=True)
                        ob = osbp.tile([P, 640], f32, tag="ob")
                        cengs = [nc.vector.tensor_copy, nc.scalar.copy]
                        cengs[(c * 2 + bt) % 2](
                            ob[:].rearrange("p (a b) -> p a b", a=2),
                            op[:, :, 0:320])
                        eng = engs[(c * 2 + bt) % 4]
                        nsteps = S if c < NCH - 1 else S - 1
                        eng.dma_start(
                            pred_d[bt * 128:(bt + 1) * 128, c * S:c * S + nsteps, :],
                            ob[:, 0:nsteps * L].rearrange("p (s l) -> p s l", l=L))

    nc.compile()
    return nc


def kernel(xs, us, W0, b0, W1, b1, W2, Bw, K):
    global _BUILT
    from concourse.bass_utils import run_bass_kernel_spmd

    if _BUILT is None:
        _BUILT = _build()
    nc = _BUILT

    CW, CWH, CB = _precompute_consts(W0, b0, W1, b1, W2, Bw, K)
    xs = np.ascontiguousarray(np.asarray(xs, np.float32))
    us = np.ascontiguousarray(np.asarray(us, np.float32))

    in_maps = []
    for k in range(NCORES):
        sl = slice(k * BC, (k + 1) * BC)
        in_maps.append({"xs": xs[sl], "us": us[sl], "CW": CW, "CWH": CWH, "CB": CB})

    res = run_bass_kernel_spmd(nc, in_maps, list(range(NCORES)),
                           trace=bool(int(os.environ.get("DEINA_TRACE", "0"))))
    y2s = np.concatenate([res.results[k]["y2s"] for k in range(NCORES)], axis=0)
    pred = np.concatenate([res.results[k]["y2s_pred"] for k in range(NCORES)], axis=0)
    kernel.last_exec_time_ns = res.exec_time_ns
    kernel.last_results = res
    return y2s, pred
